# revision 3
# baseline (speedup 1.0000x reference)
"""CLIF spiking-neuron recurrence kernel for 8 Trainium2 NeuronCores.

Reference semantics (per element, T=64 sequential steps, gamma=0.5):
    u     = 0.5*u + x_t
    spike = (u >= 1.0)
    m     = s_prev * sigmoid(0.5*u) + spike
    s     = sigmoid(m)                       # carried (in-place sigmoid_)
    u     = u - spike*(1.0 + s)
Output: spikes [T, B, D] float32.

Strategy:
- Pure data parallel over the B*D = 524288 elements: 65536 per core,
  laid out as [128 partitions x 512 free], split into G=2 independent
  pipeline groups of [128 x 256] so the serial per-step dependency
  chain of one group hides under the other group's engine work.
- The membrane potential lives in PSUM as V_t = 2^t * u_t. All linear
  updates run on the (otherwise idle) TensorEngine as identity-matmul
  accumulates:  V += I @ (2^t * x_t)  and  V -= (2^t I) @ r_t  with
  r_t = spike*(1+s). Power-of-two scaling is exact in fp32, and
  2^63 * |u|max stays far below the fp32 range, so this reproduces the
  reference recurrence rounding-for-rounding.
- spike is derived from sg = sigmoid(2^-(t+1) * V) by a threshold
  compare against c = sigmoidLUT(0.5) computed on-device: the ACT LUT
  is strictly monotone around z=0.5 (verified on HW), so
  (sg >= c) <=> (u >= 1) exactly. This saves a third ACT op per step.
- Per step+group: ACT does 2 sigmoids, DVE does one compare (2x mode),
  one multiply and two fused scalar_tensor_tensor ops, PE does 2-4
  identity matmuls. The m-state assembly alternates between DVE (group
  0) and PE (group 1) to balance engine load.
- Output spikes are written as bf16 (0/1 exact) to halve the output
  bandwidth; the host maps them back to float32.
"""

import numpy as np
import ml_dtypes
import concourse.bass as bass
import concourse.bacc as bacc
import concourse.mybir as mybir
import concourse.tile as tile
from concourse.bass_utils import run_bass_kernel_spmd

F32 = mybir.dt.float32
BF16 = mybir.dt.bfloat16
AF = mybir.ActivationFunctionType
ALU = mybir.AluOpType

T = 64
B = 128
D = 4096
N_CORES = 8
P = 128
NPC = B * D // N_CORES          # 65536 elements per core
FDT = NPC // P                  # 512 free columns per core
G = 2                           # pipeline groups
FD = FDT // G                   # 256 free columns per group

_NC_CACHE = None
LAST_RESULTS = None


def _build():
    nc = bacc.Bacc(None, target_bir_lowering=False, debug=False,
                   num_devices=N_CORES)

    xs = nc.declare_dram_parameter("xs", [T, G, P, FD], F32, isOutput=False)
    # weight table, SBUF layout [P, T+1, P]: slot 0 = I, slot 1+t = -(2^t) I
    wt = nc.declare_dram_parameter("wt", [P, T + 1, P], F32, isOutput=False)
    wtb = nc.declare_dram_parameter("wtb", [P, P], BF16, isOutput=False)  # I bf16
    out = nc.declare_dram_parameter("out", [T, G, P, FD], BF16, isOutput=True)

    with tile.TileContext(nc) as tc:
        with (
            tc.tile_pool(name="wpool", bufs=1) as wpool,
            tc.tile_pool(name="cpool", bufs=1) as cpool,
            tc.tile_pool(name="xpool", bufs=6) as xpool,
            tc.tile_pool(name="sgpool", bufs=3) as sgpool,
            tc.tile_pool(name="kpool", bufs=3) as kpool,
            tc.tile_pool(name="spool", bufs=3) as spool,
            tc.tile_pool(name="qpool", bufs=2) as qpool,
            tc.tile_pool(name="mpool", bufs=2) as mpool,
            tc.tile_pool(name="rpool", bufs=2) as rpool,
            tc.tile_pool(name="vpool", bufs=1, space="PSUM") as vpool,
            tc.tile_pool(name="mppool", bufs=2, space="PSUM") as mppool,
        ):
            # --- one-time setup -------------------------------------------
            wtile = wpool.tile([P, T + 1, P], F32, tag="wt")
            nc.sync.dma_start(wtile[:, 0:1, :], wt[:, 0:1, :])
            nc.sync.dma_start(wtile[:, 1:, :], wt[:, 1:, :])
            wbtile = wpool.tile([P, P], BF16, tag="wtb")
            nc.sync.dma_start(wbtile[:], wtb[:])

            halft = cpool.tile([P, 1], F32, tag="half")
            nc.gpsimd.memset(halft[:], 0.5)
            ct = cpool.tile([P, 1], F32, tag="c")
            # c = sigmoid_LUT(0.5), computed with the same LUT used below
            nc.scalar.activation(ct[:], halft[:], AF.Sigmoid, bias=0.0, scale=1.0)

            eye32 = wtile[:, 0, :]

            # --- initial state --------------------------------------------
            V = []
            s_prev = []
            for g in range(G):
                s0 = spool.tile([P, FD], F32, tag=f"s{g}")
                nc.gpsimd.memset(s0[:], 0.0)
                s_prev.append(s0)
                vt = vpool.tile([P, FD], F32, tag=f"V{g}")
                V.append(vt)

            xcur = []
            for g in range(G):
                xt = xpool.tile([P, FD], F32, tag=f"x{g}")
                nc.sync.dma_start(xt[:], xs[0, g])
                nc.tensor.matmul(V[g][:], eye32, xt[:], start=True, stop=False,
                                 skip_group_check=True)
                xcur.append(xt)

            # --- the recurrence -------------------------------------------
            for t in range(T):
                sc_sg = float(2.0 ** (-t - 1))
                for g in range(G):
                    # sg = sigmoid(0.5 * u_t)  (exact power-of-two input scale)
                    sg = sgpool.tile([P, FD], F32, tag=f"sg{g}")
                    nc.scalar.activation(sg[:], V[g][:], AF.Sigmoid,
                                         bias=0.0, scale=sc_sg)

                    # spike = (sg >= c)  ->  bf16 0/1, also the kernel output
                    spk = kpool.tile([P, FD], BF16, tag=f"spk{g}")
                    nc.vector.tensor_scalar(spk[:], sg[:], ct[:, 0:1], None,
                                            op0=ALU.is_ge)
                    nc.sync.dma_start(out[t, g], spk[:])

                    if t == T - 1:
                        continue  # m/s/V updates past the last step are dead

                    # q = s_prev * sg
                    q = qpool.tile([P, FD], F32, tag=f"q{g}")
                    nc.vector.tensor_mul(q[:], s_prev[g][:], sg[:])

                    # m = q + spike ; s = sigmoid(m)
                    s_new = spool.tile([P, FD], F32, tag=f"s{g}")
                    if g == 0:
                        msb = mpool.tile([P, FD], F32, tag="msb")
                        nc.vector.scalar_tensor_tensor(
                            msb[:], sg[:], ct[:, 0:1], q[:],
                            op0=ALU.is_ge, op1=ALU.add)
                        nc.scalar.activation(s_new[:], msb[:], AF.Sigmoid,
                                             bias=0.0, scale=1.0)
                    else:
                        mp = mppool.tile([P, FD], F32, tag="mp")
                        nc.tensor.matmul(mp[:], eye32, q[:],
                                         start=True, stop=False)
                        nc.tensor.matmul(mp[:], wbtile[:], spk[:],
                                         start=False, stop=True)
                        nc.scalar.activation(s_new[:], mp[:], AF.Sigmoid,
                                             bias=0.0, scale=1.0)
                    s_prev[g] = s_new

                    # r = (s + 1) * spike ; V -= 2^t * r ; V += 2^(t+1) x_(t+1)
                    r = rpool.tile([P, FD], F32, tag=f"r{g}")
                    nc.vector.scalar_tensor_tensor(r[:], s_new[:], 1.0, spk[:],
                                                   op0=ALU.add, op1=ALU.mult)
                    nc.tensor.matmul(V[g][:], wtile[:, 1 + t, :], r[:],
                                     start=False, stop=False,
                                     skip_group_check=True)
                    xt = xpool.tile([P, FD], F32, tag=f"x{g}")
                    nc.sync.dma_start(xt[:], xs[t + 1, g])
                    nc.tensor.matmul(V[g][:], eye32, xt[:],
                                     start=False, stop=(t + 1 == T - 1),
                                     skip_group_check=True)
                    xcur[g] = xt

    nc.compile()
    return nc


def _get_nc():
    global _NC_CACHE
    if _NC_CACHE is None:
        _NC_CACHE = _build()
    return _NC_CACHE


def kernel(x_seq: np.ndarray) -> np.ndarray:
    global LAST_RESULTS
    x = np.ascontiguousarray(x_seq, dtype=np.float32)
    assert x.shape == (T, B, D), x.shape

    # 2^t prescale (exact in fp32) and per-core shard [T, G, P, FD]
    scale = (2.0 ** np.arange(T, dtype=np.float64)).astype(np.float32)
    xsc = x.reshape(T, -1) * scale[:, None]
    xsc = xsc.reshape(T, N_CORES, G, P, FD)

    # weight table in SBUF layout [P, T+1, P]
    wt_host = np.zeros((P, T + 1, P), dtype=np.float32)
    idx = np.arange(P)
    wt_host[idx, 0, idx] = 1.0
    for t in range(T):
        wt_host[idx, 1 + t, idx] = -float(2.0 ** t)
    wtb_host = np.eye(P, dtype=np.float32).astype(ml_dtypes.bfloat16)

    nc = _get_nc()
    in_maps = [
        {"xs": np.ascontiguousarray(xsc[:, c]), "wt": wt_host, "wtb": wtb_host}
        for c in range(N_CORES)
    ]
    LAST_RESULTS = run_bass_kernel_spmd(nc, in_maps, list(range(N_CORES)))

    full = np.empty((T, N_CORES, G, P, FD), dtype=np.float32)
    for c in range(N_CORES):
        full[:, c] = np.asarray(LAST_RESULTS.results[c]["out"], dtype=np.float32)
    return full.reshape(T, B, D)
